# revision 65
# baseline (speedup 1.0000x reference)
"""Bahdanau-attention scores kernel for Trainium2, 8-core data-parallel.

Computes softmax_s( v . tanh(W_h @ h[b] + W_e @ enc[s,b] + bias) ) for
B=32, S=2048, Dd=512, De2=1024, sharded 4 batches per NeuronCore.

v5 design:
  Pass 1 (fp8): E^T = W_e8 @ enc8 on the PE in e4m3 DoubleRow mode,
    j-major groups of 8 matmuls into 2-bank [128,1024] PSUM tiles with
    a ring of THREE tiles (6 banks) so the group's bank-release WAR
    (tanh of group j-3) never stalls the PE at the 216ns matmul pitch.
    One [128,1024] tanh per group on ACT (bias + 1/128 dequant), then
    a fused multiply-accumulate chain on DVE (scalar_tensor_tensor)
    forms prodacc = sum_j v_j * tanh_j.
  Scores: block t's two [1,512] score chunks are reduced over
    partitions by ones-matmuls at the head and middle of block t+1
    into a single shared PSUM bank (temporally split so the WAR on the
    PSUM->SBUF copy never blocks), then copied into per-batch
    [1,2048] score rows (h0 via ACT, h1 via DVE).
  Select: top-8 of each 512-chunk via an SBUF->SBUF DMA relayout to
    [4,512] + max_with_indices (32 candidates/batch, rel err 2.4e-3
    measured against the dataset vs the 2e-2 gate).
  Refine (fp16): batches in pairs: per-batch 32-row gathers (indirect
    DMA, base-0 [32,1] index tiles), PE transposes, 8 stationary-
    gather matmuls of N=512 with the weights as the moving operand,
    h-projection bias folded in as a 9th matmul against a constant
    batch-selector, tanh, v-row multiply + free-dim reduce, exp.
  Merge: denominator = ACT accum_out row sums of exp(screen) minus
    exp(screen@selected) plus exp(refined@selected); refined
    probabilities overwrite the base row via an indirect scatter.

The h-projection (hidden @ W_h^T + bias) is precomputed on host in
exact f32 and shipped as a per-partition bias table.
"""

import os

import numpy as np

KSTAGE = int(os.environ.get("KSTAGE", "6"))

B = 32
S = 2048
DD = 512
DE2 = 1024
NCORES = 8
BL = B // NCORES  # 4 batches per core
R = BL * S  # 8192 rows per core
NK = DE2 // 128  # 8 k-chunks
NO = DD // 128  # 4 o-chunks
NB2 = R // 1024  # 8 DMA blocks of 1024 rows
EXP_OFF = -26.0  # softmax shift; scores observed in [-32, 27]
W8SCALE = 128.0  # fp8 weight pre-scale (keeps W_e out of e4m3 subnormals)
NWARM = int(os.environ.get("NWARM", "165"))

_CACHE = {}


def _build_bass():
    import concourse.bacc as bacc
    import concourse.mybir as mybir
    import concourse.tile as tile
    import concourse.bass as bass
    from concourse._compat import get_trn_type

    f32 = mybir.dt.float32
    f16 = mybir.dt.float16
    f8 = mybir.dt.float8e4
    i32 = mybir.dt.int32
    u32 = mybir.dt.uint32
    AF = mybir.ActivationFunctionType
    DR = mybir.MatmulPerfMode.DoubleRow

    nc = bacc.Bacc(get_trn_type() or "TRN2", target_bir_lowering=False, debug=False)

    encB8 = nc.dram_tensor("encB8", [128, NB2 * NK * 1024], f8, kind="ExternalInput")
    w8 = nc.dram_tensor("w8", [128, NO * NK * 128], f8, kind="ExternalInput")
    w16 = nc.dram_tensor("w16", [128, NK * 512], f16, kind="ExternalInput")
    hb_in = nc.dram_tensor("hb_in", [128, NO * BL], f32, kind="ExternalInput")
    v_pb = nc.dram_tensor("v_pb", [128, NO], f32, kind="ExternalInput")
    encP16 = nc.dram_tensor("encP16", [R, DE2], f16, kind="ExternalInput")
    ident16 = nc.dram_tensor("ident16", [128, 128], f16, kind="ExternalInput")
    hbp_in = nc.dram_tensor("hbp_in", [2, 2 * 512], f16, kind="ExternalInput")
    selpair_in = nc.dram_tensor("selpair_in", [2, 64], f16, kind="ExternalInput")
    v64_in = nc.dram_tensor("v64_in", [64, 512], f16, kind="ExternalInput")
    posg_in = nc.dram_tensor("posg_in", [4, BL], f32, kind="ExternalInput")
    probs = nc.dram_tensor("probs", [R, 1], f32, kind="ExternalOutput")

    with tile.TileContext(nc) as tc:
        with (
            tc.tile_pool(name="const", bufs=1) as const,
            tc.tile_pool(name="encp", bufs=3) as encp,
            tc.tile_pool(name="etp", bufs=8) as etp,
            tc.tile_pool(name="prp", bufs=10) as prp,
            tc.tile_pool(name="refp", bufs=4) as refp,
            tc.tile_pool(name="pep", bufs=6, space="PSUM") as pep,
            tc.tile_pool(name="psc", bufs=1, space="PSUM") as psc,
            tc.tile_pool(name="pref", bufs=1, space="PSUM") as pref,
        ):
            # ---- PE warm-up: dummy matmuls while DMAs stream in ----
            warm_sb = const.tile([128, 128], f16, name="warm_sb")
            nc.any.memset(warm_sb[:], 0.0)
            wu_ps = pref.tile([128, 128], f32, name="wu_ps", tag="rf2")
            for _ in range(NWARM):
                nc.tensor.matmul(wu_ps[:], warm_sb[:], warm_sb[:], start=True, stop=True)

            # ---- constants / weights (scalar queue) ----
            encB_v = encB8[:].rearrange("p (t k r) -> p t k r", t=NB2, k=NK)
            w8_sb = const.tile([128, NO, NK, 128], f8, name="w8_sb")
            w8_v = w8[:].rearrange("p (j k oo) -> p j k oo", j=NO, k=NK)
            for j in range(NO):
                nc.scalar.dma_start(w8_sb[:, j], w8_v[:, j])
            hb_sb = const.tile([128, NO, BL], f32, name="hb_sb")
            nc.scalar.dma_start(hb_sb[:], hb_in[:].rearrange("p (j b) -> p j b", j=NO))
            v_sb = const.tile([128, NO], f32, name="v_sb")
            nc.scalar.dma_start(v_sb[:], v_pb[:])
            # w16_sb[p, k, jo] = W_e[jo, 128k+p]: contiguous 512-slab per k
            w16_sb = const.tile([128, NK, 512], f16, name="w16_sb")
            nc.scalar.dma_start(
                w16_sb[:], w16[:].rearrange("p (k jo) -> p k jo", k=NK)
            )
            id_sb = const.tile([128, 128], f16, name="id_sb")
            nc.scalar.dma_start(id_sb[:], ident16[:])
            hbp_sb = const.tile([2, 2 * 512], f16, name="hbp_sb")
            nc.scalar.dma_start(hbp_sb[:], hbp_in[:])
            selpair = const.tile([2, 64], f16, name="selpair")
            nc.scalar.dma_start(selpair[:], selpair_in[:])
            v64_sb = const.tile([64, 512], f16, name="v64_sb")
            nc.scalar.dma_start(v64_sb[:], v64_in[:])
            posg = const.tile([4, BL], f32, name="posg")
            nc.scalar.dma_start(posg[:], posg_in[:])

            ones_v = const.tile([128, 1], f16, name="ones_v")
            nc.any.memset(ones_v[:], 1.0)
            ones4 = const.tile([4, 1], f32, name="ones4")
            nc.any.memset(ones4[:], 1.0)
            mones4 = const.tile([4, 1], f32, name="mones4")
            nc.any.memset(mones4[:], -1.0)
            o32c = const.tile([32, 1], f32, name="o32c")
            nc.any.memset(o32c[:], 1.0)
            o14 = const.tile([1, 4], f32, name="o14")
            nc.any.memset(o14[:], 1.0)
            o132 = const.tile([1, 32], f32, name="o132")
            nc.any.memset(o132[:], 1.0)
            zero64 = const.tile([64, 1], f32, name="zero64")
            nc.any.memset(zero64[:], 0.0)
            nopt = const.tile([1, 1], f32, name="nopt")
            nc.any.memset(nopt[:], 0.0)
            nopt2 = const.tile([1, 1], f32, name="nopt2")
            nc.any.memset(nopt2[:], 0.0)
            expoff = const.tile([64, 1], f32, name="expoff")
            nc.any.memset(expoff[:], EXP_OFF)
            scrow = [const.tile([1, S], f32, name=f"scrow{b}") for b in range(BL)]

            probs_flat = probs[:]
            probs_row_v = probs[:].rearrange("(b p t) one -> b p (t one)", b=BL, p=4)

            # ---- refine stages; st holds cross-stage tiles ----
            st = {}

            def p1_relayout(b):
                # scores [1,2048] -> [4,512] via SBUF->SBUF DMA
                sc4 = refp.tile([4, 512], f32, name="sc4", tag="sc4")
                # sync-queue trigger: keeps the DMA-launch instruction off
                # the ACT queue, whose progress semaphore gates PE PSUM reuse
                nc.sync.dma_start(sc4[:], scrow[b][:])
                st[("sc4", b)] = sc4

            def p23_select(b):
                sc4 = st[("sc4", b)]
                m8 = refp.tile([4, 8], f32, name="m8", tag="m8")
                mi = refp.tile([4, 8], u32, name="mi", tag="mi")
                nc.vector.max_with_indices(m8[:], mi[:], sc4[:])
                mif = refp.tile([4, 8], f32, name="mif", tag="mif")
                nc.vector.tensor_copy(mif[:], mi[:])
                gidxf = refp.tile([4, 8], f32, name="gidxf", tag="gxf")
                nc.vector.tensor_scalar(
                    gidxf[:], mif[:], posg[:, b : b + 1], None, mybir.AluOpType.add
                )
                gidxi = refp.tile([4, 8], i32, name="gidxi", tag="gxi")
                nc.vector.tensor_copy(gidxi[:], gidxf[:])
                st[("m8", b)] = m8
                st[("gidxi", b)] = gidxi

            def p4_idx(b, q, r, use_sync):
                # [4,8] -> a base-0 [32,1] index tile (the indirect-DMA
                # offset AP must not have a partition base offset)
                idx32 = refp.tile([32, 1], i32, name="idx32", tag=f"i32{r}")
                eng = nc.sync if use_sync else nc.scalar
                eng.dma_start(idx32[:], st[("gidxi", b)][:])
                st[("idx32", q, r)] = idx32

            def p5_expo(b):
                # exp of the fp8 screen row + rowsums; exp of the selected
                # screen maxima (for the denominator correction)
                sc4 = st[("sc4", b)]
                expo = refp.tile([4, 512], f32, name="expo", tag="expo")
                rows4 = refp.tile([4, 1], f32, name="rows4", tag="rw4")
                nc.scalar.activation(
                    expo[:], sc4[:], AF.Exp, bias=expoff[:4], accum_out=rows4[:]
                )
                em8 = refp.tile([4, 8], f32, name="em8", tag="em8")
                em8s = refp.tile([4, 1], f32, name="em8s", tag="em8s")
                nc.scalar.activation(
                    em8[:], st[("m8", b)][:], AF.Exp, bias=expoff[:4], accum_out=em8s[:]
                )
                st[("expo", b)] = expo
                st[("rows4", b)] = rows4
                st[("em8s", b)] = em8s

            def p6_gather(q, r):
                # 32-row gather of fp16 enc rows for one batch of the pair
                gath = refp.tile([32, DE2], f16, name="gath", tag=f"gath{r}")
                nc.gpsimd.indirect_dma_start(
                    out=gath[:],
                    out_offset=None,
                    in_=encP16[:],
                    in_offset=bass.IndirectOffsetOnAxis(
                        ap=st[("idx32", q, r)][:, :1], axis=0
                    ),
                )
                st[("gath", q, r)] = gath

            def p7_transpose(q, r):
                # [32,1024] -> [128, NK, 32] slices of the pair's encsel
                if ("encsel", q) not in st:
                    st[("encsel", q)] = refp.tile(
                        [128, NK, 64], f16, name="encsel", tag="esel"
                    )
                encsel = st[("encsel", q)]
                gath = st.pop(("gath", q, r))
                for k in range(NK):
                    tp = pref.tile([128, 32], f16, name="tp", tag="rf2")
                    nc.tensor.transpose(
                        tp[:], gath[:, 128 * k : 128 * (k + 1)], id_sb[:32, :32]
                    )
                    nc.vector.tensor_copy(encsel[:, k, 32 * r : 32 * (r + 1)], tp[:])

            def p8_ematmul(q):
                # E[cand, oo] for all 64 cands: stationary gathered enc,
                # moving fp16 weights; bias row via the selpair matmul
                encsel = st.pop(("encsel", q))
                rpe = pref.tile([64, 512], f32, name="rpe", tag="rf2")
                for k in range(NK):
                    nc.tensor.matmul(
                        rpe[:],
                        encsel[:, k, :],
                        w16_sb[:, k, :],
                        start=(k == 0),
                        stop=False,
                    )
                nc.tensor.matmul(
                    rpe[:],
                    selpair[:],
                    hbp_sb[:, 512 * q : 512 * (q + 1)],
                    start=False,
                    stop=True,
                )
                st[("rpe", q)] = rpe

            def p9_12_scores(q):
                rpe = st.pop(("rpe", q))
                rt = refp.tile([64, 512], f16, name="rt", tag="rt")
                nc.scalar.activation(rt[:], rpe[:], AF.Tanh, bias=zero64[:])
                vm = refp.tile([64, 512], f16, name="vm", tag="vm")
                nc.vector.tensor_mul(vm[:], rt[:], v64_sb[:])
                sref = refp.tile([64, 1], f32, name="sref", tag="sref")
                nc.vector.reduce_sum(sref[:], vm[:], axis=mybir.AxisListType.X)
                es = refp.tile([64, 1], f32, name="es", tag="es")
                nc.scalar.activation(es[:], sref[:], AF.Exp, bias=expoff[:])
                es2 = refp.tile([32, 2], f32, name="es2", tag="es2")
                nc.scalar.dma_start(es2[:, 0:1], es[0:32, :])
                nc.sync.dma_start(es2[:, 1:2], es[32:64, :])
                st[("es", q)] = es
                st[("es2", q)] = es2

            def p13_finalize(q, r):
                b = 2 * q + r
                es2 = st[("es2", q)]
                idx32 = st[("idx32", q, r)]
                # merged denominator: sum(expo) - sum(exp(screen@sel)) + sum(exp(ref@sel))
                dtot = pref.tile([1, 1], f32, name="dtot", tag="rf2")
                nc.tensor.matmul(
                    dtot[:], ones4[:], st.pop(("rows4", b))[:], start=True, stop=False
                )
                nc.tensor.matmul(
                    dtot[:], mones4[:], st.pop(("em8s", b))[:], start=False, stop=False
                )
                nc.tensor.matmul(
                    dtot[:], o32c[:], es2[:, r : r + 1], start=False, stop=True
                )
                totS = refp.tile([1, 1], f32, name="totS", tag="totS")
                nc.vector.tensor_copy(totS[:], dtot[:])
                rec = refp.tile([1, 1], f32, name="rec", tag="rec")
                nc.vector.reciprocal(rec[:], totS[:])
                rb4ps = pref.tile([4, 1], f32, name="rb4ps", tag="rf2")
                nc.tensor.matmul(rb4ps[:], o14[:], rec[:], start=True, stop=True)
                rb4 = refp.tile([4, 1], f32, name="rb4", tag="rb4")
                nc.vector.tensor_copy(rb4[:], rb4ps[:])
                rb32ps = pref.tile([32, 1], f32, name="rb32ps", tag="rf2")
                nc.tensor.matmul(rb32ps[:], o132[:], rec[:], start=True, stop=True)
                rb32 = refp.tile([32, 1], f32, name="rb32", tag="rb32")
                nc.vector.tensor_copy(rb32[:], rb32ps[:])
                probs4 = refp.tile([4, 512], f32, name="probs4", tag="p4")
                nc.vector.tensor_scalar_mul(probs4[:], st.pop(("expo", b))[:], rb4[:])
                pr32 = refp.tile([32, 1], f32, name="pr32", tag="pr32")
                nc.vector.tensor_scalar_mul(pr32[:], es2[:, r : r + 1], rb32[:])
                # base row on the sync queue; the overlapping indirect
                # scatter is ordered after it by the DRAM range tracking
                nc.sync.dma_start(probs_row_v[b], probs4[:])
                nc.gpsimd.indirect_dma_start(
                    out=probs_flat,
                    out_offset=bass.IndirectOffsetOnAxis(ap=idx32[:, :1], axis=0),
                    in_=pr32[:],
                    in_offset=None,
                )

            # block-end hook schedule (deps are >= 1 block old); the stage
            # number gates hardware bisection via KSTAGE
            raw_hooks = {
                2: [(1, lambda: p1_relayout(0))],
                3: [(2, lambda: p23_select(0))],
                4: [
                    (2, lambda: p4_idx(0, 0, 0, use_sync=True)),
                    (1, lambda: p1_relayout(1)),
                ],
                5: [(4, lambda: p6_gather(0, 0)), (2, lambda: p23_select(1))],
                6: [
                    (4, lambda: p7_transpose(0, 0)),
                    (2, lambda: p4_idx(1, 0, 1, use_sync=True)),
                    (1, lambda: p1_relayout(2)),
                ],
                7: [
                    (4, lambda: p6_gather(0, 1)),
                    (2, lambda: p23_select(2)),
                    (2, lambda: p4_idx(2, 1, 0, use_sync=True)),
                ],
                8: [
                    (4, lambda: p6_gather(1, 0)),
                    (3, lambda: p5_expo(0)),
                    (3, lambda: p5_expo(1)),
                    (1, lambda: p1_relayout(3)),
                    (4, lambda: p7_transpose(0, 1)),
                    (5, lambda: p8_ematmul(0)),
                    (3, lambda: p5_expo(2)),
                    (5, lambda: p9_12_scores(0)),
                    (2, lambda: p23_select(3)),
                    (3, lambda: p5_expo(3)),
                    (2, lambda: p4_idx(3, 1, 1, use_sync=False)),
                    (4, lambda: p6_gather(1, 1)),
                    (4, lambda: p7_transpose(1, 0)),
                    (4, lambda: p7_transpose(1, 1)),
                    (5, lambda: p8_ematmul(1)),
                    (6, lambda: p13_finalize(0, 0)),
                    (6, lambda: p13_finalize(0, 1)),
                    (5, lambda: p9_12_scores(1)),
                    (6, lambda: p13_finalize(1, 0)),
                    (6, lambda: p13_finalize(1, 1)),
                ],
            }
            hooks = {
                t: [f for (s, f) in fns if s <= KSTAGE] for t, fns in raw_hooks.items()
            }
            if KSTAGE < 6:
                probs_b_rows = probs[:].rearrange("(b s) one -> b (s one)", b=BL)
                hooks.setdefault(8, []).extend(
                    (lambda bb=bb: nc.gpsimd.dma_start(
                        probs_b_rows[bb : bb + 1, :], scrow[bb][:]
                    ))
                    for bb in range(BL)
                )

            # ---- main loop ----
            def emit_sc(t2, h, prev_pa):
                # partition-reduce one [1,512] score chunk of block t2-1
                # into the single shared PSUM bank, then copy to scrow
                sc = psc.tile([1, 512], f32, name="sc", tag="sc")
                nc.tensor.matmul(
                    sc[:], ones_v[:], prev_pa[h][:], start=True, stop=True
                )
                bprev = (t2 - 1) // 2
                t_i = ((t2 - 1) % 2) * 2 + h
                dst = scrow[bprev][0:1, 512 * t_i : 512 * (t_i + 1)]
                # DVE copy: the ACT queue stays pure-tanh in the main loop
                nc.vector.tensor_copy(dst, sc[:])

            prev_pa = None
            for t2 in range(NB2 + 1):
                if t2 < NB2:
                    enc_t = encp.tile([128, NK, 1024], f8, name="enc_t", tag="enc")
                    nc.sync.dma_start(enc_t[:], encB_v[:, t2])
                if prev_pa is not None:
                    emit_sc(t2, 0, prev_pa)
                    if t2 == NB2:
                        emit_sc(t2, 1, prev_pa)
                if t2 < NB2:
                    b = t2 // 2
                    pa_h = [None, None]
                    for h in range(2):
                        for j in range(NO):
                            if (h, j) == (1, 1) and prev_pa is not None:
                                # h1 score chunk mid-block: the shared sc
                                # bank's WAR (h0's copy) is long done
                                emit_sc(t2, 1, prev_pa)
                            pe = pep.tile([128, 512], f32, name="pe", tag="pe")
                            for kk in range(NK // 2):
                                nc.tensor.matmul(
                                    pe[:],
                                    w8_sb[:, j, 2 * kk : 2 * kk + 2, :],
                                    enc_t[:, 2 * kk : 2 * kk + 2, 512 * h : 512 * (h + 1)],
                                    start=(kk == 0),
                                    stop=(kk == NK // 2 - 1),
                                    perf_mode=DR,
                                )
                            et = etp.tile([128, 512], f16, name="et", tag="et")
                            nc.scalar.activation(
                                et[:],
                                pe[:],
                                AF.Tanh,
                                bias=hb_sb[:, j, b : b + 1],
                                scale=1.0 / W8SCALE,
                            )
                            if j == 0:
                                pa = prp.tile([128, 512], f16, name="pa", tag="pa")
                                nc.vector.tensor_scalar_mul(
                                    pa[:], et[:], v_sb[:, 0:1]
                                )
                            else:
                                pa2 = prp.tile([128, 512], f16, name="pa2", tag="pa")
                                nc.vector.scalar_tensor_tensor(
                                    pa2[:],
                                    et[:],
                                    v_sb[:, j : j + 1],
                                    pa[:],
                                    mybir.AluOpType.mult,
                                    mybir.AluOpType.add,
                                )
                                pa = pa2
                        pa_h[h] = pa
                    prev_pa = pa_h
                else:
                    prev_pa = None
                for fn in hooks.get(t2, []):
                    fn()

    nc.compile()
    return nc


def _get_nc():
    if "nc" not in _CACHE:
        _CACHE["nc"] = _build_bass()
    return _CACHE["nc"]


def _tile_rows(mat_t, nchunk):
    # [nchunk*128, F] -> [128, nchunk*F] with out[p, c*F+f] = mat_t[128c+p, f]
    n, F = mat_t.shape
    assert n == nchunk * 128
    return np.ascontiguousarray(
        mat_t.reshape(nchunk, 128, F).transpose(1, 0, 2)
    ).reshape(128, nchunk * F)


def _make_in_maps(hidden, enc, W, b, v):
    import ml_dtypes

    f8 = ml_dtypes.float8_e4m3
    W_h = W[:, :DD]
    W_e = W[:, DD:]
    # w8[p, j, k, oo] = W_e[128j+oo, 128k+p]
    w_lay = np.ascontiguousarray(
        W_e.reshape(NO, 128, NK, 128).transpose(3, 0, 2, 1)
    ).reshape(128, NO * NK * 128)
    w8_arr = (w_lay * W8SCALE).astype(f8)
    # w16[p, (k, jo)] = W_e[jo, 128k+p]
    w16_arr = np.ascontiguousarray(
        W_e.reshape(DD, NK, 128).transpose(2, 1, 0)
    ).reshape(128, NK * DD).astype(np.float16)
    v_pb = np.ascontiguousarray(v.reshape(NO, 128).T).astype(np.float32)
    ident = np.eye(128, dtype=np.float16)
    v64 = np.broadcast_to(v.astype(np.float16), (64, DD)).copy()
    selpair = np.zeros((2, 64), dtype=np.float16)
    selpair[0, 0:32] = 1.0
    selpair[1, 32:64] = 1.0
    posg = (
        2048.0 * np.arange(BL)[None, :] + 512.0 * np.arange(4)[:, None]
    ).astype(np.float32)
    in_maps = []
    for c in range(NCORES):
        ec = enc[:, BL * c : BL * (c + 1), :]  # [S, BL, DE2]
        encT = np.ascontiguousarray(ec.transpose(2, 1, 0)).reshape(DE2, R)
        encB = np.ascontiguousarray(
            encT.reshape(NK, 128, NB2, 1024).transpose(1, 2, 0, 3)
        ).reshape(128, NB2 * NK * 1024)
        encB8 = encB.astype(f8)
        encP16 = np.ascontiguousarray(ec.transpose(1, 0, 2)).reshape(R, DE2).astype(
            np.float16
        )
        # exact f32 h-projection + bias, tiled per-partition: [128, (j, b)]
        h_proj = hidden[BL * c : BL * (c + 1), :] @ W_h.T + b  # [BL, DD]
        hb = _tile_rows(np.ascontiguousarray(h_proj.T), NO)  # [128, NO*BL]
        # pair bias rows: hbp[r, 512q+oo] = h_proj[2q+r, oo]
        hbp = np.ascontiguousarray(
            h_proj.reshape(2, 2, DD).transpose(1, 0, 2)
        ).reshape(2, 2 * DD).astype(np.float16)
        in_maps.append(
            {
                "encB8": encB8,
                "w8": w8_arr,
                "w16": w16_arr,
                "hb_in": np.ascontiguousarray(hb, dtype=np.float32),
                "v_pb": v_pb,
                "encP16": encP16,
                "ident16": ident,
                "hbp_in": hbp,
                "selpair_in": selpair,
                "v64_in": v64,
                "posg_in": posg,
            }
        )
    return in_maps


def kernel(hidden, encoder_outputs, W, b, v):
    """Full inputs in, full output out; 8-way batch-parallel inside."""
    from concourse.bass_utils import run_bass_kernel_spmd

    hidden = np.asarray(hidden, dtype=np.float32)
    enc = np.asarray(encoder_outputs, dtype=np.float32)
    W = np.asarray(W, dtype=np.float32)
    b = np.asarray(b, dtype=np.float32)
    v = np.asarray(v, dtype=np.float32)

    in_maps = _make_in_maps(hidden, enc, W, b, v)
    nc = _get_nc()
    res = run_bass_kernel_spmd(nc, in_maps, core_ids=list(range(NCORES)))
    out = np.concatenate(
        [res.results[c]["probs"].reshape(BL, S) for c in range(NCORES)], axis=0
    )
    return out.astype(np.float32)


# revision 66
# speedup vs baseline: 1.0171x; 1.0171x over previous
"""Bahdanau-attention scores kernel for Trainium2, 8-core data-parallel.

Computes softmax_s( v . tanh(W_h @ h[b] + W_e @ enc[s,b] + bias) ) for
B=32, S=2048, Dd=512, De2=1024, sharded 4 batches per NeuronCore.

v5 design:
  Pass 1 (fp8): E^T = W_e8 @ enc8 on the PE in e4m3 DoubleRow mode,
    j-major groups of 8 matmuls into 2-bank [128,1024] PSUM tiles with
    a ring of THREE tiles (6 banks) so the group's bank-release WAR
    (tanh of group j-3) never stalls the PE at the 216ns matmul pitch.
    One [128,1024] tanh per group on ACT (bias + 1/128 dequant), then
    a fused multiply-accumulate chain on DVE (scalar_tensor_tensor)
    forms prodacc = sum_j v_j * tanh_j.
  Scores: block t's two [1,512] score chunks are reduced over
    partitions by ones-matmuls at the head and middle of block t+1
    into a single shared PSUM bank (temporally split so the WAR on the
    PSUM->SBUF copy never blocks), then copied into per-batch
    [1,2048] score rows (h0 via ACT, h1 via DVE).
  Select: top-8 of each 512-chunk via an SBUF->SBUF DMA relayout to
    [4,512] + max_with_indices (32 candidates/batch, rel err 2.4e-3
    measured against the dataset vs the 2e-2 gate).
  Refine (fp16): batches in pairs: per-batch 32-row gathers (indirect
    DMA, base-0 [32,1] index tiles), PE transposes, 8 stationary-
    gather matmuls of N=512 with the weights as the moving operand,
    h-projection bias folded in as a 9th matmul against a constant
    batch-selector, tanh, v-row multiply + free-dim reduce, exp.
  Merge: denominator = ACT accum_out row sums of exp(screen) minus
    exp(screen@selected) plus exp(refined@selected); refined
    probabilities overwrite the base row via an indirect scatter.

The h-projection (hidden @ W_h^T + bias) is precomputed on host in
exact f32 and shipped as a per-partition bias table.
"""

import os

import numpy as np

KSTAGE = int(os.environ.get("KSTAGE", "6"))

B = 32
S = 2048
DD = 512
DE2 = 1024
NCORES = 8
BL = B // NCORES  # 4 batches per core
R = BL * S  # 8192 rows per core
NK = DE2 // 128  # 8 k-chunks
NO = DD // 128  # 4 o-chunks
NB2 = R // 1024  # 8 DMA blocks of 1024 rows
EXP_OFF = -26.0  # softmax shift; scores observed in [-32, 27]
W8SCALE = 128.0  # fp8 weight pre-scale (keeps W_e out of e4m3 subnormals)
NWARM = int(os.environ.get("NWARM", "165"))

_CACHE = {}


def _build_bass():
    import concourse.bacc as bacc
    import concourse.mybir as mybir
    import concourse.tile as tile
    import concourse.bass as bass
    from concourse._compat import get_trn_type

    f32 = mybir.dt.float32
    f16 = mybir.dt.float16
    f8 = mybir.dt.float8e4
    i32 = mybir.dt.int32
    u32 = mybir.dt.uint32
    AF = mybir.ActivationFunctionType
    DR = mybir.MatmulPerfMode.DoubleRow

    nc = bacc.Bacc(get_trn_type() or "TRN2", target_bir_lowering=False, debug=False)

    encB8 = nc.dram_tensor("encB8", [128, NB2 * NK * 1024], f8, kind="ExternalInput")
    w8 = nc.dram_tensor("w8", [128, NO * NK * 128], f8, kind="ExternalInput")
    w16 = nc.dram_tensor("w16", [128, NK * 512], f16, kind="ExternalInput")
    hb_in = nc.dram_tensor("hb_in", [128, NO * BL], f32, kind="ExternalInput")
    v_pb = nc.dram_tensor("v_pb", [128, NO], f32, kind="ExternalInput")
    encP16 = nc.dram_tensor("encP16", [R, DE2], f16, kind="ExternalInput")
    ident16 = nc.dram_tensor("ident16", [128, 128], f16, kind="ExternalInput")
    hbp_in = nc.dram_tensor("hbp_in", [2, 2 * 512], f16, kind="ExternalInput")
    selpair_in = nc.dram_tensor("selpair_in", [2, 64], f16, kind="ExternalInput")
    v64_in = nc.dram_tensor("v64_in", [64, 512], f16, kind="ExternalInput")
    posg_in = nc.dram_tensor("posg_in", [4, BL], f32, kind="ExternalInput")
    probs = nc.dram_tensor("probs", [R, 1], f32, kind="ExternalOutput")

    with tile.TileContext(nc) as tc:
        with (
            tc.tile_pool(name="const", bufs=1) as const,
            tc.tile_pool(name="encp", bufs=3) as encp,
            tc.tile_pool(name="etp", bufs=8) as etp,
            tc.tile_pool(name="prp", bufs=10) as prp,
            tc.tile_pool(name="refp", bufs=4) as refp,
            tc.tile_pool(name="pep", bufs=6, space="PSUM") as pep,
            tc.tile_pool(name="psc", bufs=1, space="PSUM") as psc,
            tc.tile_pool(name="pref", bufs=1, space="PSUM") as pref,
        ):
            # ---- PE warm-up: dummy matmuls while DMAs stream in ----
            warm_sb = const.tile([128, 128], f16, name="warm_sb")
            nc.any.memset(warm_sb[:], 0.0)
            wu_ps = pref.tile([128, 128], f32, name="wu_ps", tag="rf2")
            for _ in range(NWARM):
                nc.tensor.matmul(wu_ps[:], warm_sb[:], warm_sb[:], start=True, stop=True)

            # ---- constants / weights (scalar queue) ----
            encB_v = encB8[:].rearrange("p (t k r) -> p t k r", t=NB2, k=NK)
            w8_sb = const.tile([128, NO, NK, 128], f8, name="w8_sb")
            w8_v = w8[:].rearrange("p (j k oo) -> p j k oo", j=NO, k=NK)
            for j in range(NO):
                nc.scalar.dma_start(w8_sb[:, j], w8_v[:, j])
            hb_sb = const.tile([128, NO, BL], f32, name="hb_sb")
            nc.scalar.dma_start(hb_sb[:], hb_in[:].rearrange("p (j b) -> p j b", j=NO))
            v_sb = const.tile([128, NO], f32, name="v_sb")
            nc.scalar.dma_start(v_sb[:], v_pb[:])
            # w16_sb[p, k, jo] = W_e[jo, 128k+p]: contiguous 512-slab per k
            w16_sb = const.tile([128, NK, 512], f16, name="w16_sb")
            nc.scalar.dma_start(
                w16_sb[:], w16[:].rearrange("p (k jo) -> p k jo", k=NK)
            )
            id_sb = const.tile([128, 128], f16, name="id_sb")
            nc.scalar.dma_start(id_sb[:], ident16[:])
            hbp_sb = const.tile([2, 2 * 512], f16, name="hbp_sb")
            nc.scalar.dma_start(hbp_sb[:], hbp_in[:])
            selpair = const.tile([2, 64], f16, name="selpair")
            nc.scalar.dma_start(selpair[:], selpair_in[:])
            v64_sb = const.tile([64, 512], f16, name="v64_sb")
            nc.scalar.dma_start(v64_sb[:], v64_in[:])
            posg = const.tile([4, BL], f32, name="posg")
            nc.scalar.dma_start(posg[:], posg_in[:])

            ones_v = const.tile([128, 1], f16, name="ones_v")
            nc.any.memset(ones_v[:], 1.0)
            ones4 = const.tile([4, 1], f32, name="ones4")
            nc.any.memset(ones4[:], 1.0)
            mones4 = const.tile([4, 1], f32, name="mones4")
            nc.any.memset(mones4[:], -1.0)
            o32c = const.tile([32, 1], f32, name="o32c")
            nc.any.memset(o32c[:], 1.0)
            o14 = const.tile([1, 4], f32, name="o14")
            nc.any.memset(o14[:], 1.0)
            o132 = const.tile([1, 32], f32, name="o132")
            nc.any.memset(o132[:], 1.0)
            zero64 = const.tile([64, 1], f32, name="zero64")
            nc.any.memset(zero64[:], 0.0)
            nopt = const.tile([1, 1], f32, name="nopt")
            nc.any.memset(nopt[:], 0.0)
            nopt2 = const.tile([1, 1], f32, name="nopt2")
            nc.any.memset(nopt2[:], 0.0)
            expoff = const.tile([64, 1], f32, name="expoff")
            nc.any.memset(expoff[:], EXP_OFF)
            scrow = [const.tile([1, S], f32, name=f"scrow{b}") for b in range(BL)]

            probs_flat = probs[:]
            probs_row_v = probs[:].rearrange("(b p t) one -> b p (t one)", b=BL, p=4)

            # ---- refine stages; st holds cross-stage tiles ----
            st = {}

            def p1_relayout(b):
                # scores [1,2048] -> [4,512] via SBUF->SBUF DMA
                sc4 = refp.tile([4, 512], f32, name="sc4", tag="sc4")
                # sync-queue trigger: keeps the DMA-launch instruction off
                # the ACT queue, whose progress semaphore gates PE PSUM reuse
                nc.sync.dma_start(sc4[:], scrow[b][:])
                st[("sc4", b)] = sc4

            def p23_select(b):
                sc4 = st[("sc4", b)]
                m8 = refp.tile([4, 8], f32, name="m8", tag="m8")
                mi = refp.tile([4, 8], u32, name="mi", tag="mi")
                nc.vector.max_with_indices(m8[:], mi[:], sc4[:])
                mif = refp.tile([4, 8], f32, name="mif", tag="mif")
                nc.vector.tensor_copy(mif[:], mi[:])
                gidxf = refp.tile([4, 8], f32, name="gidxf", tag="gxf")
                nc.vector.tensor_scalar(
                    gidxf[:], mif[:], posg[:, b : b + 1], None, mybir.AluOpType.add
                )
                gidxi = refp.tile([4, 8], i32, name="gidxi", tag="gxi")
                nc.vector.tensor_copy(gidxi[:], gidxf[:])
                st[("m8", b)] = m8
                st[("gidxi", b)] = gidxi

            def p4_idx(b, q, r, use_sync):
                # [4,8] -> a base-0 [32,1] index tile (the indirect-DMA
                # offset AP must not have a partition base offset)
                idx32 = refp.tile([32, 1], i32, name="idx32", tag=f"i32{r}")
                eng = nc.sync if use_sync else nc.scalar
                eng.dma_start(idx32[:], st[("gidxi", b)][:])
                st[("idx32", q, r)] = idx32

            def p5_expo(b):
                # exp of the fp8 screen row + rowsums; exp of the selected
                # screen maxima (for the denominator correction)
                sc4 = st[("sc4", b)]
                expo = refp.tile([4, 512], f32, name="expo", tag="expo")
                rows4 = refp.tile([4, 1], f32, name="rows4", tag="rw4")
                nc.scalar.activation(
                    expo[:], sc4[:], AF.Exp, bias=expoff[:4], accum_out=rows4[:]
                )
                em8 = refp.tile([4, 8], f32, name="em8", tag="em8")
                em8s = refp.tile([4, 1], f32, name="em8s", tag="em8s")
                nc.scalar.activation(
                    em8[:], st[("m8", b)][:], AF.Exp, bias=expoff[:4], accum_out=em8s[:]
                )
                st[("expo", b)] = expo
                st[("rows4", b)] = rows4
                st[("em8s", b)] = em8s

            def p6_gather(q, r):
                # 32-row gather of fp16 enc rows for one batch of the pair
                gath = refp.tile([32, DE2], f16, name="gath", tag=f"gath{r}")
                nc.gpsimd.indirect_dma_start(
                    out=gath[:],
                    out_offset=None,
                    in_=encP16[:],
                    in_offset=bass.IndirectOffsetOnAxis(
                        ap=st[("idx32", q, r)][:, :1], axis=0
                    ),
                )
                st[("gath", q, r)] = gath

            def p7_transpose(q, r):
                # [32,1024] -> [128, NK, 32] slices of the pair's encsel
                if ("encsel", q) not in st:
                    st[("encsel", q)] = refp.tile(
                        [128, NK, 64], f16, name="encsel", tag="esel"
                    )
                encsel = st[("encsel", q)]
                gath = st.pop(("gath", q, r))
                for k in range(NK):
                    tp = pref.tile([128, 32], f16, name="tp", tag="rf2")
                    nc.tensor.transpose(
                        tp[:], gath[:, 128 * k : 128 * (k + 1)], id_sb[:32, :32]
                    )
                    nc.vector.tensor_copy(encsel[:, k, 32 * r : 32 * (r + 1)], tp[:])

            def p8_ematmul(q):
                # E[cand, oo] for all 64 cands: stationary gathered enc,
                # moving fp16 weights; bias row via the selpair matmul
                encsel = st.pop(("encsel", q))
                rpe = pref.tile([64, 512], f32, name="rpe", tag="rf2")
                for k in range(NK):
                    nc.tensor.matmul(
                        rpe[:],
                        encsel[:, k, :],
                        w16_sb[:, k, :],
                        start=(k == 0),
                        stop=False,
                    )
                nc.tensor.matmul(
                    rpe[:],
                    selpair[:],
                    hbp_sb[:, 512 * q : 512 * (q + 1)],
                    start=False,
                    stop=True,
                )
                st[("rpe", q)] = rpe

            def p9_12_scores(q):
                rpe = st.pop(("rpe", q))
                rt = refp.tile([64, 512], f16, name="rt", tag="rt")
                nc.scalar.activation(rt[:], rpe[:], AF.Tanh, bias=zero64[:])
                vm = refp.tile([64, 512], f16, name="vm", tag="vm")
                nc.vector.tensor_mul(vm[:], rt[:], v64_sb[:])
                sref = refp.tile([64, 1], f32, name="sref", tag="sref")
                nc.vector.reduce_sum(sref[:], vm[:], axis=mybir.AxisListType.X)
                es = refp.tile([64, 1], f32, name="es", tag="es")
                nc.scalar.activation(es[:], sref[:], AF.Exp, bias=expoff[:])
                es2 = refp.tile([32, 2], f32, name="es2", tag="es2")
                nc.scalar.dma_start(es2[:, 0:1], es[0:32, :])
                nc.sync.dma_start(es2[:, 1:2], es[32:64, :])
                st[("es", q)] = es
                st[("es2", q)] = es2

            def p13_finalize(q, r):
                b = 2 * q + r
                es2 = st[("es2", q)]
                idx32 = st[("idx32", q, r)]
                # merged denominator: sum(expo) - sum(exp(screen@sel)) + sum(exp(ref@sel))
                dtot = pref.tile([1, 1], f32, name="dtot", tag="rf2")
                nc.tensor.matmul(
                    dtot[:], ones4[:], st.pop(("rows4", b))[:], start=True, stop=False
                )
                nc.tensor.matmul(
                    dtot[:], mones4[:], st.pop(("em8s", b))[:], start=False, stop=False
                )
                nc.tensor.matmul(
                    dtot[:], o32c[:], es2[:, r : r + 1], start=False, stop=True
                )
                totS = refp.tile([1, 1], f32, name="totS", tag="totS")
                nc.vector.tensor_copy(totS[:], dtot[:])
                rec = refp.tile([1, 1], f32, name="rec", tag="rec")
                nc.vector.reciprocal(rec[:], totS[:])
                rb4ps = pref.tile([4, 1], f32, name="rb4ps", tag="rf2")
                nc.tensor.matmul(rb4ps[:], o14[:], rec[:], start=True, stop=True)
                rb4 = refp.tile([4, 1], f32, name="rb4", tag="rb4")
                nc.vector.tensor_copy(rb4[:], rb4ps[:])
                rb32ps = pref.tile([32, 1], f32, name="rb32ps", tag="rf2")
                nc.tensor.matmul(rb32ps[:], o132[:], rec[:], start=True, stop=True)
                rb32 = refp.tile([32, 1], f32, name="rb32", tag="rb32")
                nc.vector.tensor_copy(rb32[:], rb32ps[:])
                probs4 = refp.tile([4, 512], f32, name="probs4", tag="p4")
                nc.vector.tensor_scalar_mul(probs4[:], st.pop(("expo", b))[:], rb4[:])
                pr32 = refp.tile([32, 1], f32, name="pr32", tag="pr32")
                nc.vector.tensor_scalar_mul(pr32[:], es2[:, r : r + 1], rb32[:])
                # base row on the sync queue; the overlapping indirect
                # scatter is ordered after it by the DRAM range tracking
                nc.sync.dma_start(probs_row_v[b], probs4[:])
                nc.gpsimd.indirect_dma_start(
                    out=probs_flat,
                    out_offset=bass.IndirectOffsetOnAxis(ap=idx32[:, :1], axis=0),
                    in_=pr32[:],
                    in_offset=None,
                )

            # block-end hook schedule (deps are >= 1 block old); the stage
            # number gates hardware bisection via KSTAGE
            raw_hooks = {
                2: [(1, lambda: p1_relayout(0))],
                3: [(2, lambda: p23_select(0))],
                4: [
                    (2, lambda: p4_idx(0, 0, 0, use_sync=True)),
                    (1, lambda: p1_relayout(1)),
                ],
                5: [(4, lambda: p6_gather(0, 0)), (2, lambda: p23_select(1))],
                6: [
                    (4, lambda: p7_transpose(0, 0)),
                    (2, lambda: p4_idx(1, 0, 1, use_sync=True)),
                    (1, lambda: p1_relayout(2)),
                ],
                7: [
                    (4, lambda: p6_gather(0, 1)),
                    (2, lambda: p23_select(2)),
                    (2, lambda: p4_idx(2, 1, 0, use_sync=True)),
                ],
                8: [
                    (4, lambda: p6_gather(1, 0)),
                    (3, lambda: p5_expo(0)),
                    (3, lambda: p5_expo(1)),
                    (1, lambda: p1_relayout(3)),
                    (4, lambda: p7_transpose(0, 1)),
                    (5, lambda: p8_ematmul(0)),
                    (3, lambda: p5_expo(2)),
                    (5, lambda: p9_12_scores(0)),
                    (2, lambda: p23_select(3)),
                    (3, lambda: p5_expo(3)),
                    (2, lambda: p4_idx(3, 1, 1, use_sync=False)),
                    (4, lambda: p6_gather(1, 1)),
                    (4, lambda: p7_transpose(1, 0)),
                    (4, lambda: p7_transpose(1, 1)),
                    (5, lambda: p8_ematmul(1)),
                    (6, lambda: p13_finalize(0, 0)),
                    (6, lambda: p13_finalize(0, 1)),
                    (5, lambda: p9_12_scores(1)),
                    (6, lambda: p13_finalize(1, 0)),
                    (6, lambda: p13_finalize(1, 1)),
                ],
            }
            hooks = {
                t: [f for (s, f) in fns if s <= KSTAGE] for t, fns in raw_hooks.items()
            }
            if KSTAGE < 6:
                probs_b_rows = probs[:].rearrange("(b s) one -> b (s one)", b=BL)
                hooks.setdefault(8, []).extend(
                    (lambda bb=bb: nc.gpsimd.dma_start(
                        probs_b_rows[bb : bb + 1, :], scrow[bb][:]
                    ))
                    for bb in range(BL)
                )

            # ---- main loop ----
            def emit_sc(t2, h, prev_pa):
                # partition-reduce one [1,512] score chunk of block t2-1
                # into the single shared PSUM bank, then copy to scrow
                sc = psc.tile([1, 512], f32, name="sc", tag="sc")
                nc.tensor.matmul(
                    sc[:], ones_v[:], prev_pa[h][:], start=True, stop=True
                )
                bprev = (t2 - 1) // 2
                t_i = ((t2 - 1) % 2) * 2 + h
                dst = scrow[bprev][0:1, 512 * t_i : 512 * (t_i + 1)]
                nc.scalar.copy(dst, sc[:])

            prev_pa = None
            for t2 in range(NB2 + 1):
                if t2 < NB2:
                    enc_t = encp.tile([128, NK, 1024], f8, name="enc_t", tag="enc")
                    nc.sync.dma_start(enc_t[:], encB_v[:, t2])
                if prev_pa is not None:
                    emit_sc(t2, 0, prev_pa)
                    if t2 == NB2:
                        emit_sc(t2, 1, prev_pa)
                if t2 < NB2:
                    b = t2 // 2
                    pa_h = [None, None]
                    for h in range(2):
                        for j in range(NO):
                            if (h, j) == (1, 1) and prev_pa is not None:
                                # h1 score chunk mid-block: the shared sc
                                # bank's WAR (h0's copy) is long done
                                emit_sc(t2, 1, prev_pa)
                            pe = pep.tile([128, 512], f32, name="pe", tag="pe")
                            for kk in range(NK // 2):
                                nc.tensor.matmul(
                                    pe[:],
                                    w8_sb[:, j, 2 * kk : 2 * kk + 2, :],
                                    enc_t[:, 2 * kk : 2 * kk + 2, 512 * h : 512 * (h + 1)],
                                    start=(kk == 0),
                                    stop=(kk == NK // 2 - 1),
                                    perf_mode=DR,
                                )
                            et = etp.tile([128, 512], f16, name="et", tag="et")
                            nc.scalar.activation(
                                et[:],
                                pe[:],
                                AF.Tanh,
                                bias=hb_sb[:, j, b : b + 1],
                                scale=1.0 / W8SCALE,
                            )
                            if j == 0:
                                pa = prp.tile([128, 512], f16, name="pa", tag="pa")
                                nc.vector.tensor_scalar_mul(
                                    pa[:], et[:], v_sb[:, 0:1]
                                )
                            else:
                                pa2 = prp.tile([128, 512], f16, name="pa2", tag="pa")
                                nc.vector.scalar_tensor_tensor(
                                    pa2[:],
                                    et[:],
                                    v_sb[:, j : j + 1],
                                    pa[:],
                                    mybir.AluOpType.mult,
                                    mybir.AluOpType.add,
                                )
                                pa = pa2
                        pa_h[h] = pa
                    prev_pa = pa_h
                else:
                    prev_pa = None
                for fn in hooks.get(t2, []):
                    fn()

    nc.compile()
    return nc


def _get_nc():
    if "nc" not in _CACHE:
        _CACHE["nc"] = _build_bass()
    return _CACHE["nc"]


def _tile_rows(mat_t, nchunk):
    # [nchunk*128, F] -> [128, nchunk*F] with out[p, c*F+f] = mat_t[128c+p, f]
    n, F = mat_t.shape
    assert n == nchunk * 128
    return np.ascontiguousarray(
        mat_t.reshape(nchunk, 128, F).transpose(1, 0, 2)
    ).reshape(128, nchunk * F)


def _make_in_maps(hidden, enc, W, b, v):
    import ml_dtypes

    f8 = ml_dtypes.float8_e4m3
    W_h = W[:, :DD]
    W_e = W[:, DD:]
    # w8[p, j, k, oo] = W_e[128j+oo, 128k+p]
    w_lay = np.ascontiguousarray(
        W_e.reshape(NO, 128, NK, 128).transpose(3, 0, 2, 1)
    ).reshape(128, NO * NK * 128)
    w8_arr = (w_lay * W8SCALE).astype(f8)
    # w16[p, (k, jo)] = W_e[jo, 128k+p]
    w16_arr = np.ascontiguousarray(
        W_e.reshape(DD, NK, 128).transpose(2, 1, 0)
    ).reshape(128, NK * DD).astype(np.float16)
    v_pb = np.ascontiguousarray(v.reshape(NO, 128).T).astype(np.float32)
    ident = np.eye(128, dtype=np.float16)
    v64 = np.broadcast_to(v.astype(np.float16), (64, DD)).copy()
    selpair = np.zeros((2, 64), dtype=np.float16)
    selpair[0, 0:32] = 1.0
    selpair[1, 32:64] = 1.0
    posg = (
        2048.0 * np.arange(BL)[None, :] + 512.0 * np.arange(4)[:, None]
    ).astype(np.float32)
    in_maps = []
    for c in range(NCORES):
        ec = enc[:, BL * c : BL * (c + 1), :]  # [S, BL, DE2]
        encT = np.ascontiguousarray(ec.transpose(2, 1, 0)).reshape(DE2, R)
        encB = np.ascontiguousarray(
            encT.reshape(NK, 128, NB2, 1024).transpose(1, 2, 0, 3)
        ).reshape(128, NB2 * NK * 1024)
        encB8 = encB.astype(f8)
        encP16 = np.ascontiguousarray(ec.transpose(1, 0, 2)).reshape(R, DE2).astype(
            np.float16
        )
        # exact f32 h-projection + bias, tiled per-partition: [128, (j, b)]
        h_proj = hidden[BL * c : BL * (c + 1), :] @ W_h.T + b  # [BL, DD]
        hb = _tile_rows(np.ascontiguousarray(h_proj.T), NO)  # [128, NO*BL]
        # pair bias rows: hbp[r, 512q+oo] = h_proj[2q+r, oo]
        hbp = np.ascontiguousarray(
            h_proj.reshape(2, 2, DD).transpose(1, 0, 2)
        ).reshape(2, 2 * DD).astype(np.float16)
        in_maps.append(
            {
                "encB8": encB8,
                "w8": w8_arr,
                "w16": w16_arr,
                "hb_in": np.ascontiguousarray(hb, dtype=np.float32),
                "v_pb": v_pb,
                "encP16": encP16,
                "ident16": ident,
                "hbp_in": hbp,
                "selpair_in": selpair,
                "v64_in": v64,
                "posg_in": posg,
            }
        )
    return in_maps


def kernel(hidden, encoder_outputs, W, b, v):
    """Full inputs in, full output out; 8-way batch-parallel inside."""
    from concourse.bass_utils import run_bass_kernel_spmd

    hidden = np.asarray(hidden, dtype=np.float32)
    enc = np.asarray(encoder_outputs, dtype=np.float32)
    W = np.asarray(W, dtype=np.float32)
    b = np.asarray(b, dtype=np.float32)
    v = np.asarray(v, dtype=np.float32)

    in_maps = _make_in_maps(hidden, enc, W, b, v)
    nc = _get_nc()
    res = run_bass_kernel_spmd(nc, in_maps, core_ids=list(range(NCORES)))
    out = np.concatenate(
        [res.results[c]["probs"].reshape(BL, S) for c in range(NCORES)], axis=0
    )
    return out.astype(np.float32)
